# revision 77
# baseline (speedup 1.0000x reference)
"""PASA group-softmax high-pass downsample kernel for 8 Trainium2 NeuronCores.

Reference computation (n=4, c=64, h=w=128, G=2 groups, K=3, stride 2):
  xp     = reflect_pad(x, 1)
  sigma  = conv3x3(xp, conv_w)                    # [n, 18, h, w]
  sigma  = sigma * bn_scale + bn_shift            # BN (inference)
  sigma  = softmax(sigma, axis=1)                 # over all 18 channels
  sigma  = onehot(center) - sigma                 # high-pass
  out[n,g,c,i,j] = sum_k patches[n,g,c,k,i,j] * sigma[n,g,k,i,j]
  return out[:, :, ::2, ::2]                      # [4, 64, 64, 64]

Sharding: core = (image n, h-half).  Each core computes 2048 output
positions x 64 channels; the two 16-row sub-halves (s=A/B) are stacked on
SBUF partitions (p = 64*s + c) so all ops use 128 lanes.

Key optimizations over the previous version:
  - x is host-packed into 4 parity planes P[a,b][r,w] = xpad[2r+a, 2w+b],
    so every stride-2 patch/conv access becomes a unit-stride view.
  - One packed constants DMA; 5 input DMAs total (was 11+).
  - Compact conv: 2 column-half matmuls per tap (h folded into the
    contraction), concurrent on the PE via tile_position.
  - softmax: exp -> D (matmul) -> reciprocal_approx_fast (1 op) -> -1/D
    broadcast (negated rsel) -> f = -E/D in bf16.
  - F broadcast to channel layout via ONE shifted selector matrix (the tap
    shift is applied by slicing the rhs partition window f[k:k+120]).
  - 6/9 taps stage Fbig through ScalarE into SBUF bf16 so the DVE multiply
    runs in 2x mode; the 3 dx=2 taps (misaligned anyway) read PSUM direct.
  - The center pixel is added into the PSUM accumulator with an identity
    matmul, so y = xc - sum(F*patch) needs no final vector subtract;
    output is stored as bf16 and upcast on the host.
  - PE warm-up matmuls run on memset garbage (no DMA dependency) so the
    HAM clock ungates before the conv.
"""

import os
import ml_dtypes
import numpy as np

import concourse.bass as bass
import concourse.tile as tile
from concourse import bacc, mybir
from concourse.bass_utils import run_bass_kernel_spmd

F32 = mybir.dt.float32
BF16 = mybir.dt.bfloat16

N, C, H, W = 4, 64, 128, 128
G, K = 2, 3
K2 = K * K
EPS = 1e-5
NCORES = 8
HO, WO = H // 2, W // 2            # 64 x 64 output spatial
SUB = 16                           # output rows per sub-half (s = A/B)
PLANE_R, PLANE_W = 17, 66          # parity plane dims (padded)
POS = SUB * WO                     # 1024 positions per channel-partition
CHUNK = POS // 2                   # 512 = one PSUM bank of f32

NWARM1 = 28                        # warm matmuls before the base matmul
NWARM2 = 10                        # warm matmuls after it

# const tile column layout (bf16 columns)
WTS_C0 = 0                         # [9, 64] conv lhsT per tap
IDENT_C0 = WTS_C0 + K2 * 64        # [128] identity
SEL_C0 = IDENT_C0 + 128            # [4] D-selector
RSEL_C0 = SEL_C0 + 4               # [128] (-1) r-broadcast selector (rows 0..3)
BIAS_C0 = RSEL_C0 + 128            # [2] f32 BN shift, bitcast
PM_C0 = BIAS_C0 + 2                # [2] f32 p % 64 per partition, bitcast
BASE_C0 = PM_C0 + 2                # [128] row 0: 32*(j//64) + 9*((j//32)%2)
ONE_C0 = BASE_C0 + 128             # [128] row 0: 1.0 (ones lhsT)
CONST_COLS = ONE_C0 + 128 + 2      # padded to keep the row pitch even

CONV_ORDER = [(dy, dx) for dy in range(K) for dx in range(K)]

_compiled = None


def _build_program():
    nc = bacc.Bacc(
        "TRN2", target_bir_lowering=False, debug=False, num_devices=NCORES
    )

    xplanes = nc.dram_tensor(
        "xplanes", [128, PLANE_R, 4, PLANE_W], BF16, kind="ExternalInput"
    )
    consts = nc.dram_tensor("consts", [128, CONST_COLS], BF16,
                            kind="ExternalInput")
    y = nc.dram_tensor("y", [128, POS], BF16, kind="ExternalOutput")
    warm_out = nc.dram_tensor("warm_out", [1, 2], F32, kind="ExternalOutput")

    with tile.TileContext(nc) as tc:
        with (
            tc.tile_pool(name="singles", bufs=1) as singles,
            tc.tile_pool(name="psacc", bufs=1, space="PSUM") as psacc,
            tc.tile_pool(name="pbig", bufs=3, space="PSUM") as pbig,
            tc.tile_pool(name="fb", bufs=4) as fb_pool,
            tc.tile_pool(name="prods", bufs=5) as prod_pool,
            tc.tile_pool(name="work", bufs=3) as work,
        ):
            # ---- DMA issues (5 input DMAs across 3 rings) ----
            const_sb = singles.tile([128, CONST_COLS], BF16)
            nc.sync.dma_start(const_sb[:], consts.ap())
            xp_sb = singles.tile([128, PLANE_R, 4, PLANE_W], BF16)
            # two fat-descriptor DMAs; rows 0..8 land first so the ch0 half
            # of the conv can start while rows 9..16 stream in
            nc.scalar.dma_start(xp_sb[:, 0:9], xplanes.ap()[:, 0:9])
            nc.scalar.dma_start(xp_sb[:, 9:PLANE_R], xplanes.ap()[:, 9:PLANE_R])
            junk = work.tile([128, 128], BF16, tag="junk")
            nc.vector.memset(junk[:], 0.0)
            warm_in = work.tile([1, 1], F32, tag="warm_in")
            nc.vector.memset(warm_in[:], 0.25)

            # prewarm ACT's exp table (overlaps the x DMA)
            warm_e = work.tile([1, 1], F32, tag="warm")
            nc.scalar.activation(warm_e[:], warm_in[:],
                                 mybir.ActivationFunctionType.Exp)

            # const views
            wts_v = const_sb[:, WTS_C0 : WTS_C0 + K2 * 64].rearrange(
                "p (k j) -> p k j", k=K2
            )
            ident_v = const_sb[:, IDENT_C0 : IDENT_C0 + 128]
            sel_v = const_sb[:, SEL_C0 : SEL_C0 + 4]
            rsel_v = const_sb[0:4, RSEL_C0 : RSEL_C0 + 128]
            bias_v = const_sb[:, BIAS_C0 : BIAS_C0 + 2].bitcast(F32)
            pm_v = const_sb[:, PM_C0 : PM_C0 + 2].bitcast(F32)
            base_v = const_sb[0:1, BASE_C0 : BASE_C0 + 128]
            ones_v = const_sb[0:1, ONE_C0 : ONE_C0 + 128]

            # ---- PE warm-up on memset garbage (keeps HAM busy pre-conv),
            # with the selector base matmul slotted in once consts arrive ----
            warm_ps = pbig.tile([128, 128], F32, tag="pb",
                                padded_shape=[128, POS])
            NWARM = NWARM1 + NWARM2
            base_ps = None
            esel_sb = singles.tile([128, K2, 128], BF16)
            for i in range(NWARM):
                nc.tensor.matmul(warm_ps[:], junk[:], junk[:],
                                 start=(i == 0), stop=(i == NWARM - 1),
                                 skip_group_check=True)
                if i == NWARM1 - 1:
                    # base_ps[p, j] = base[j]
                    base_ps = pbig.tile([128, 128], F32, tag="pb",
                                        padded_shape=[128, POS])
                    nc.tensor.matmul(base_ps[:], ones_v, base_v)
            warm_sb = work.tile([1, 2], F32, tag="warm_sb")
            nc.vector.tensor_copy(warm_sb[:], warm_ps[0:1, 0:2])
            nc.sync.dma_start(warm_out.ap(), warm_sb[:])

            # esel[p, k, j] = ((base[j] - p%64) == -k), via a bf16 staging
            # copy so the 9 comparisons run in 4x mode
            base_sb = singles.tile([128, 128], BF16)
            nc.scalar.copy(base_sb[:], base_ps[:])
            for k in range(K2):
                nc.vector.tensor_scalar(
                    esel_sb[:, k, :], base_sb[:], pm_v, float(-k),
                    mybir.AluOpType.subtract, mybir.AluOpType.is_equal,
                )

            # ---- conv: 9 taps x 2 column-halves into one PSUM bank ----
            # sigma[32q + o, (r8, w)] with q = 2*ch + s; all ch0 matmuls
            # first (their input rows arrive in the first plane DMA)
            sigma_ps = pbig.tile([128, CHUNK], F32, tag="pb",
                                 padded_shape=[128, POS])
            for ch in range(2):
                for i, (dy, dx) in enumerate(CONV_ORDER):
                    k = 3 * dy + dx
                    ab = 2 * (dy % 2) + (dx % 2)
                    rhs = xp_sb[
                        :,
                        dy // 2 + 8 * ch : dy // 2 + 8 * ch + 8,
                        ab,
                        dx // 2 : dx // 2 + WO,
                    ]
                    nc.tensor.matmul(
                        sigma_ps[64 * ch : 64 * ch + 64, :],
                        wts_v[:, k, :],
                        rhs,
                        start=(i == 0),
                        stop=(i == K2 - 1),
                        tile_position=(0, 64 * ch),
                        skip_group_check=True,
                    )

            # ---- E = exp(sigma + bn_shift) in bf16 ----
            e_sb = singles.tile([128, CHUNK], BF16)
            nc.scalar.activation(
                e_sb[:], sigma_ps[:], mybir.ActivationFunctionType.Exp,
                bias=bias_v, scale=1.0,
            )

            # ---- f = -E/D in bf16 (rsel carries the negation) ----
            d_ps = pbig.tile([4, CHUNK], F32, tag="pb",
                             padded_shape=[128, POS])
            nc.tensor.matmul(d_ps[:], sel_v[:], e_sb[:])
            r_sb = singles.tile([4, CHUNK], F32)
            nc.vector.reciprocal_approx_fast(r_sb[:], d_ps[:])
            r_bf = singles.tile([4, CHUNK], BF16)
            nc.vector.tensor_copy(r_bf[:], r_sb[:])
            rbig_ps = pbig.tile([128, CHUNK], F32, tag="pb",
                                padded_shape=[128, POS])
            nc.tensor.matmul(rbig_ps[:], rsel_v[:], r_bf[:])
            f_sb = singles.tile([128, CHUNK], BF16)
            nc.vector.tensor_mul(f_sb[:], e_sb[:], rbig_ps[:])

            # ---- acc starts from the center pixel (identity matmul) ----
            acc_ps = [psacc.tile([128, CHUNK], F32, name=f"acc{ch}",
                                 tag=f"acc{ch}")
                      for ch in range(2)]
            for ch in range(2):
                xc = xp_sb[:, 8 * ch : 8 * ch + 8, 3, 0:WO]
                nc.tensor.matmul(
                    acc_ps[ch][:], ident_v, xc,
                    start=True, stop=False, skip_group_check=True,
                )

            # ---- apply: acc += patch_k * (-F_k) for the 9 taps ----
            def acc_mm(prod, last):
                for ch in range(2):
                    nc.tensor.matmul(
                        acc_ps[ch][:],
                        ident_v,
                        prod[:, CHUNK * ch : CHUNK * (ch + 1)],
                        start=False, stop=last, skip_group_check=True,
                    )

            # direct (dx=2) taps first in each group of three: the first
            # DVE multiply can start straight off the first broadcast
            # instead of waiting for an ACT staging copy
            APPLY_ORDER = [0, 1, 2, 3, 4, 5, 6, 7, 8]
            prods = []
            for idx, k in enumerate(APPLY_ORDER):
                dy, dx = k // K, k % K
                ab = 2 * (dy % 2) + (dx % 2)
                ebig = pbig.tile([128, POS], F32, name=f"ebig{k}",
                                 tag="pb")
                for ch in range(2):
                    # row-tiled pair: ch0 on array rows 0..63, ch1 on
                    # 64..127 -> the two matmuls run concurrently
                    nc.tensor.matmul(
                        ebig[:, CHUNK * ch : CHUNK * (ch + 1)],
                        esel_sb[64 * ch : 64 * ch + 64, k, :],
                        f_sb[64 * ch : 64 * ch + 64, :],
                        tile_position=(64 * ch, 0),
                    )
                if dx < 2:
                    # stage Fbig to SBUF bf16 on ScalarE so the DVE multiply
                    # runs in 2x mode (dx=2 taps are misaligned anyway)
                    fb = fb_pool.tile([128, POS], BF16, name=f"fb{k}",
                                      tag="fb")
                    nc.scalar.copy(fb[:], ebig[:])
                    src = fb
                else:
                    src = ebig
                patch = xp_sb[:, dy // 2 : dy // 2 + SUB, ab,
                              dx // 2 : dx // 2 + WO]
                prod = prod_pool.tile([128, POS], BF16, name=f"prod{k}",
                                      tag="prod")
                nc.vector.tensor_mul(
                    prod[:].rearrange("p (r c) -> p r c", r=SUB),
                    patch,
                    src[:].rearrange("p (r c) -> p r c", r=SUB),
                )
                prods.append(prod)
                # lag the accumulation by 2 taps so the PE queue never
                # stalls waiting on the most recent multiply
                if idx >= 2:
                    acc_mm(prods[idx - 2], last=False)
            acc_mm(prods[K2 - 2], last=False)
            acc_mm(prods[K2 - 1], last=True)

            # ---- store y = acc (bf16) ----
            y_sb0 = work.tile([128, CHUNK], BF16, tag="ysb0")
            nc.scalar.copy(y_sb0[:], acc_ps[0][:])
            nc.sync.dma_start(y.ap()[:, 0:CHUNK], y_sb0[:])
            y_sb1 = work.tile([128, CHUNK], BF16, tag="ysb1")
            nc.vector.tensor_copy(y_sb1[:], acc_ps[1][:])
            nc.sync.dma_start(y.ap()[:, CHUNK:POS], y_sb1[:])

    nc.compile()
    return nc


def _host_consts(conv_w, gamma, beta, running_mean, running_var):
    scale = gamma / np.sqrt(running_var + EPS)
    shift = beta - running_mean * scale
    w_scaled = conv_w * scale[:, None, None, None]            # [18, 64, 3, 3]

    consts = np.zeros((128, CONST_COLS), np.float32)

    # conv lhsT per tap: block-diag [[W,0],[0,W]], W = w_scaled[o, ci].T
    for dy in range(K):
        for dx in range(K):
            k = 3 * dy + dx
            Wk = w_scaled[:, :, dy, dx]                       # [18, 64]
            blk = np.zeros((128, 64), np.float32)
            blk[0:64, 0:G * K2] = Wk.T
            blk[64:128, 32 : 32 + G * K2] = Wk.T
            consts[:, WTS_C0 + 64 * k : WTS_C0 + 64 * (k + 1)] = blk

    consts[:, IDENT_C0 : IDENT_C0 + 128] = np.eye(128, dtype=np.float32)

    for q in range(4):
        consts[32 * q : 32 * q + G * K2, SEL_C0 + q] = 1.0    # D selector
        consts[q, RSEL_C0 + 32 * q : RSEL_C0 + 32 * (q + 1)] = -1.0

    # selector-generation helpers
    consts[0, BASE_C0 : BASE_C0 + 128] = [
        32 * (j // 64) + K2 * ((j // 32) % 2) for j in range(128)
    ]
    consts[0, ONE_C0 : ONE_C0 + 128] = 1.0

    cb = consts.astype(ml_dtypes.bfloat16)

    # BN shift as raw f32 bytes in two bf16 columns
    bias = np.zeros((128,), np.float32)
    for q in range(4):
        bias[32 * q : 32 * q + G * K2] = shift
    cb[:, BIAS_C0 : BIAS_C0 + 2] = (
        np.frombuffer(bias.astype("<f4").tobytes(), dtype=ml_dtypes.bfloat16)
        .reshape(128, 2)
    )
    pm = (np.arange(128) % 64).astype("<f4")
    cb[:, PM_C0 : PM_C0 + 2] = (
        np.frombuffer(pm.tobytes(), dtype=ml_dtypes.bfloat16).reshape(128, 2)
    )
    return cb


def _host_planes(x):
    """Per-core parity planes [128, 17, 4, 66] bf16 (row-major)."""
    xpad = np.pad(x, ((0, 0), (0, 0), (1, 1), (1, 1)), mode="reflect")
    planes = []
    for core in range(NCORES):
        n, half = core // 2, core % 2
        pl = np.zeros((2, C, PLANE_R, 4, PLANE_W), np.float32)
        for s in range(2):
            r0 = 64 * half + 32 * s
            for a in range(2):
                for b in range(2):
                    sl = xpad[n, :, r0 + a : r0 + 33 : 2, b : 130 : 2]
                    pl[s, :, : sl.shape[1], 2 * a + b, : sl.shape[2]] = sl
        planes.append(
            np.ascontiguousarray(pl.reshape(128, PLANE_R, 4, PLANE_W))
            .astype(ml_dtypes.bfloat16)
        )
    return planes


def _gather_output(results):
    out = np.empty((N, C, HO, WO), np.float32)
    for core, res in enumerate(results):
        n, half = core // 2, core % 2
        yc = np.asarray(res["y"], dtype=np.float32).reshape(2, C, 2, 8, WO)
        # [s, c, ch, r8, w] -> rows 32*half + 16*s + 8*ch + r8
        yc = yc.transpose(1, 0, 2, 3, 4).reshape(C, 32, WO)
        out[n, :, 32 * half : 32 * half + 32, :] = yc
    return out


def _ensure_ntff_hook():
    """Install the axon NTFF profile hook if the image's antenv lacks it."""
    try:
        from antenv import axon_hooks  # noqa: F401
        return
    except ImportError:
        pass
    try:
        import sys
        import types

        import antenv
        from trn_agent_boot.trn_boot import _ntff_profile_via_ctypes

        hook = _ntff_profile_via_ctypes("/opt/axon/libaxon_pjrt.so")
        mod = types.ModuleType("antenv.axon_hooks")
        state = {"hook": hook}
        mod.get_axon_ntff_profile_hook = lambda: state["hook"]
        mod.set_axon_ntff_profile_hook = lambda h: state.update(hook=h)
        sys.modules["antenv.axon_hooks"] = mod
        antenv.axon_hooks = mod
    except Exception:
        pass


def kernel(x, conv_w, gamma, beta, running_mean, running_var):
    global _compiled
    x = np.asarray(x, np.float32)
    conv_w = np.asarray(conv_w, np.float32)
    gamma = np.asarray(gamma, np.float32)
    beta = np.asarray(beta, np.float32)
    running_mean = np.asarray(running_mean, np.float32)
    running_var = np.asarray(running_var, np.float32)

    if _compiled is None:
        _compiled = _build_program()
    nc = _compiled

    cb = _host_consts(conv_w, gamma, beta, running_mean, running_var)
    planes = _host_planes(x)
    in_maps = [{"xplanes": planes[core], "consts": cb}
               for core in range(NCORES)]

    trace = bool(int(os.environ.get("PASA_TRACE", "0")))
    if trace:
        _ensure_ntff_hook()
    res = run_bass_kernel_spmd(
        nc, in_maps, core_ids=list(range(NCORES)), trace=trace
    )
    kernel.last_results = res
    return _gather_output(res.results)


if __name__ == "__main__":
    # quick CoreSim check of core 0 against a numpy re-implementation
    from concourse.bass_interp import CoreSim

    rng = np.random.default_rng(0)
    x = rng.standard_normal((N, C, H, W)).astype(np.float32)
    conv_w = (rng.standard_normal((G * K2, C, K, K))
              * np.sqrt(2.0 / (G * K2 * K * K))).astype(np.float32)
    gamma = rng.uniform(0.5, 1.5, G * K2).astype(np.float32)
    beta = (rng.standard_normal(G * K2) * 0.1).astype(np.float32)
    rmean = (rng.standard_normal(G * K2) * 0.1).astype(np.float32)
    rvar = rng.uniform(0.5, 1.5, G * K2).astype(np.float32)

    nc = _build_program()
    cb = _host_consts(conv_w, gamma, beta, rmean, rvar)
    planes = _host_planes(x)
    sim = CoreSim(nc)
    sim.tensor("xplanes")[:] = planes[0]
    sim.tensor("consts")[:] = cb
    sim.simulate(check_with_hw=False)
    ysim = np.asarray(sim.tensor("y"), dtype=np.float32).reshape(2, C, 2, 8, WO)
    got = ysim.transpose(1, 0, 2, 3, 4).reshape(C, 32, WO)

    # numpy reference for core 0 region (image 0, output rows 0..32)
    scale = gamma / np.sqrt(rvar + EPS)
    shift = beta - rmean * scale
    xpad = np.pad(x[0], ((0, 0), (1, 1), (1, 1)), mode="reflect")
    sig = np.zeros((G * K2, 32, WO), np.float32)
    for o in range(G * K2):
        for dy in range(K):
            for dx in range(K):
                sig[o] += np.einsum(
                    "crw->rw",
                    conv_w[o, :, dy, dx][:, None, None]
                    * xpad[:, dy : dy + 64 : 2, dx : dx + 128 : 2],
                )
    sig = sig * scale[:, None, None] + shift[:, None, None]
    e = np.exp(sig)
    r = 1.0 / e.sum(0)
    accn = np.zeros((C, 32, WO), np.float32)
    for g in range(G):
        for k in range(K2):
            dy, dx = k // K, k % K
            accn[32 * g : 32 * g + 32] += (
                xpad[32 * g : 32 * g + 32, dy : dy + 64 : 2, dx : dx + 128 : 2]
                * e[g * K2 + k][None]
            )
    ref = (xpad[:, 1:65:2, 1:129:2] - accn * r[None]).astype(np.float32)

    err = np.abs(got - ref).max() / np.abs(ref).max()
    print("sim rel err:", err)


# revision 78
# speedup vs baseline: 1.0502x; 1.0502x over previous
"""PASA group-softmax high-pass downsample kernel for 8 Trainium2 NeuronCores.

Reference computation (n=4, c=64, h=w=128, G=2 groups, K=3, stride 2):
  xp     = reflect_pad(x, 1)
  sigma  = conv3x3(xp, conv_w)                    # [n, 18, h, w]
  sigma  = sigma * bn_scale + bn_shift            # BN (inference)
  sigma  = softmax(sigma, axis=1)                 # over all 18 channels
  sigma  = onehot(center) - sigma                 # high-pass
  out[n,g,c,i,j] = sum_k patches[n,g,c,k,i,j] * sigma[n,g,k,i,j]
  return out[:, :, ::2, ::2]                      # [4, 64, 64, 64]

Sharding: core = (image n, h-half).  Each core computes 2048 output
positions x 64 channels; the two 16-row sub-halves (s=A/B) are stacked on
SBUF partitions (p = 64*s + c) so all ops use 128 lanes.

Key optimizations over the previous version:
  - x is host-packed into 4 parity planes P[a,b][r,w] = xpad[2r+a, 2w+b],
    so every stride-2 patch/conv access becomes a unit-stride view.
  - One packed constants DMA; 5 input DMAs total (was 11+).
  - Compact conv: 2 column-half matmuls per tap (h folded into the
    contraction), concurrent on the PE via tile_position.
  - softmax: exp -> D (matmul) -> reciprocal_approx_fast (1 op) -> -1/D
    broadcast (negated rsel) -> f = -E/D in bf16.
  - F broadcast to channel layout via ONE shifted selector matrix (the tap
    shift is applied by slicing the rhs partition window f[k:k+120]).
  - 6/9 taps stage Fbig through ScalarE into SBUF bf16 so the DVE multiply
    runs in 2x mode; the 3 dx=2 taps (misaligned anyway) read PSUM direct.
  - The center pixel is added into the PSUM accumulator with an identity
    matmul, so y = xc - sum(F*patch) needs no final vector subtract;
    output is stored as bf16 and upcast on the host.
  - PE warm-up matmuls run on memset garbage (no DMA dependency) so the
    HAM clock ungates before the conv.
"""

import os
import ml_dtypes
import numpy as np

import concourse.bass as bass
import concourse.tile as tile
from concourse import bacc, mybir
from concourse.bass_utils import run_bass_kernel_spmd

F32 = mybir.dt.float32
BF16 = mybir.dt.bfloat16

N, C, H, W = 4, 64, 128, 128
G, K = 2, 3
K2 = K * K
EPS = 1e-5
NCORES = 8
HO, WO = H // 2, W // 2            # 64 x 64 output spatial
SUB = 16                           # output rows per sub-half (s = A/B)
PLANE_R, PLANE_W = 17, 66          # parity plane dims (padded)
POS = SUB * WO                     # 1024 positions per channel-partition
CHUNK = POS // 2                   # 512 = one PSUM bank of f32

NWARM1 = 28                        # warm matmuls before the base matmul
NWARM2 = 10                        # warm matmuls after it

# const tile column layout (bf16 columns)
WTS_C0 = 0                         # [9, 64] conv lhsT per tap
IDENT_C0 = WTS_C0 + K2 * 64        # [128] identity
SEL_C0 = IDENT_C0 + 128            # [4] D-selector
RSEL_C0 = SEL_C0 + 4               # [128] (-1) r-broadcast selector (rows 0..3)
BIAS_C0 = RSEL_C0 + 128            # [2] f32 BN shift, bitcast
PM_C0 = BIAS_C0 + 2                # [2] f32 p % 64 per partition, bitcast
BASE_C0 = PM_C0 + 2                # [128] row 0: 32*(j//64) + 9*((j//32)%2)
ONE_C0 = BASE_C0 + 128             # [128] row 0: 1.0 (ones lhsT)
CONST_COLS = ONE_C0 + 128 + 2      # padded to keep the row pitch even

CONV_ORDER = [(dy, dx) for dy in range(K) for dx in range(K)]

_compiled = None


def _build_program():
    nc = bacc.Bacc(
        "TRN2", target_bir_lowering=False, debug=False, num_devices=NCORES
    )

    xplanes = nc.dram_tensor(
        "xplanes", [128, PLANE_R, 4, PLANE_W], BF16, kind="ExternalInput"
    )
    consts = nc.dram_tensor("consts", [128, CONST_COLS], BF16,
                            kind="ExternalInput")
    y = nc.dram_tensor("y", [128, POS], BF16, kind="ExternalOutput")
    warm_out = nc.dram_tensor("warm_out", [1, 2], F32, kind="ExternalOutput")

    with tile.TileContext(nc) as tc:
        with (
            tc.tile_pool(name="singles", bufs=1) as singles,
            tc.tile_pool(name="psacc", bufs=1, space="PSUM") as psacc,
            tc.tile_pool(name="pbig", bufs=3, space="PSUM") as pbig,
            tc.tile_pool(name="fb", bufs=4) as fb_pool,
            tc.tile_pool(name="prods", bufs=5) as prod_pool,
            tc.tile_pool(name="work", bufs=3) as work,
        ):
            # ---- DMA issues (5 input DMAs across 3 rings) ----
            const_sb = singles.tile([128, CONST_COLS], BF16)
            nc.sync.dma_start(const_sb[:], consts.ap())
            xp_sb = singles.tile([128, PLANE_R, 4, PLANE_W], BF16)
            # two fat-descriptor DMAs; rows 0..8 land first so the ch0 half
            # of the conv can start while rows 9..16 stream in
            nc.scalar.dma_start(xp_sb[:, 0:9], xplanes.ap()[:, 0:9])
            nc.scalar.dma_start(xp_sb[:, 9:PLANE_R], xplanes.ap()[:, 9:PLANE_R])
            junk = work.tile([128, 128], BF16, tag="junk")
            nc.vector.memset(junk[:], 0.0)
            warm_in = work.tile([1, 1], F32, tag="warm_in")
            nc.vector.memset(warm_in[:], 0.25)

            # prewarm ACT's exp table (overlaps the x DMA)
            warm_e = work.tile([1, 1], F32, tag="warm")
            nc.scalar.activation(warm_e[:], warm_in[:],
                                 mybir.ActivationFunctionType.Exp)

            # const views
            wts_v = const_sb[:, WTS_C0 : WTS_C0 + K2 * 64].rearrange(
                "p (k j) -> p k j", k=K2
            )
            ident_v = const_sb[:, IDENT_C0 : IDENT_C0 + 128]
            sel_v = const_sb[:, SEL_C0 : SEL_C0 + 4]
            rsel_v = const_sb[0:4, RSEL_C0 : RSEL_C0 + 128]
            bias_v = const_sb[:, BIAS_C0 : BIAS_C0 + 2].bitcast(F32)
            pm_v = const_sb[:, PM_C0 : PM_C0 + 2].bitcast(F32)
            base_v = const_sb[0:1, BASE_C0 : BASE_C0 + 128]
            ones_v = const_sb[0:1, ONE_C0 : ONE_C0 + 128]

            # ---- PE warm-up on memset garbage (keeps HAM busy pre-conv),
            # with the selector base matmul slotted in once consts arrive ----
            warm_ps = pbig.tile([128, 128], F32, tag="pb",
                                padded_shape=[128, POS])
            NWARM = NWARM1 + NWARM2
            base_ps = None
            esel_sb = singles.tile([128, K2, 128], BF16)
            for i in range(NWARM):
                nc.tensor.matmul(warm_ps[:], junk[:], junk[:],
                                 start=(i == 0), stop=(i == NWARM - 1),
                                 skip_group_check=True)
                if i == NWARM1 - 1:
                    # base_ps[p, j] = base[j]
                    base_ps = pbig.tile([128, 128], F32, tag="pb",
                                        padded_shape=[128, POS])
                    nc.tensor.matmul(base_ps[:], ones_v, base_v)
            warm_sb = work.tile([1, 2], F32, tag="warm_sb")
            nc.vector.tensor_copy(warm_sb[:], warm_ps[0:1, 0:2])
            nc.sync.dma_start(warm_out.ap(), warm_sb[:])

            # esel[p, k, j] = ((base[j] - p%64) == -k), via a bf16 staging
            # copy so the 9 comparisons run in 4x mode
            base_sb = singles.tile([128, 128], BF16)
            nc.scalar.copy(base_sb[:], base_ps[:])
            for k in range(K2):
                nc.vector.tensor_scalar(
                    esel_sb[:, k, :], base_sb[:], pm_v, float(-k),
                    mybir.AluOpType.subtract, mybir.AluOpType.is_equal,
                )

            # ---- conv: 9 taps x 2 column-halves into one PSUM bank ----
            # sigma[32q + o, (r8, w)] with q = 2*ch + s; all ch0 matmuls
            # first (their input rows arrive in the first plane DMA)
            sigma_ps = pbig.tile([128, CHUNK], F32, tag="pb",
                                 padded_shape=[128, POS])
            for ch in range(2):
                for i, (dy, dx) in enumerate(CONV_ORDER):
                    k = 3 * dy + dx
                    ab = 2 * (dy % 2) + (dx % 2)
                    rhs = xp_sb[
                        :,
                        dy // 2 + 8 * ch : dy // 2 + 8 * ch + 8,
                        ab,
                        dx // 2 : dx // 2 + WO,
                    ]
                    nc.tensor.matmul(
                        sigma_ps[64 * ch : 64 * ch + 64, :],
                        wts_v[:, k, :],
                        rhs,
                        start=(i == 0),
                        stop=(i == K2 - 1),
                        tile_position=(0, 64 * ch),
                        skip_group_check=True,
                    )

            # ---- E = exp(sigma + bn_shift) in bf16 ----
            e_sb = singles.tile([128, CHUNK], BF16)
            nc.scalar.activation(
                e_sb[:], sigma_ps[:], mybir.ActivationFunctionType.Exp,
                bias=bias_v, scale=1.0,
            )

            # ---- f = -E/D in bf16 (rsel carries the negation) ----
            d_ps = pbig.tile([4, CHUNK], F32, tag="pb",
                             padded_shape=[128, POS])
            nc.tensor.matmul(d_ps[:], sel_v[:], e_sb[:])
            r_sb = singles.tile([4, CHUNK], F32)
            nc.vector.reciprocal_approx_fast(r_sb[:], d_ps[:])
            r_bf = singles.tile([4, CHUNK], BF16)
            nc.vector.tensor_copy(r_bf[:], r_sb[:])
            rbig_ps = pbig.tile([128, CHUNK], F32, tag="pb",
                                padded_shape=[128, POS])
            nc.tensor.matmul(rbig_ps[:], rsel_v[:], r_bf[:])
            f_sb = singles.tile([128, CHUNK], BF16)
            nc.vector.tensor_mul(f_sb[:], e_sb[:], rbig_ps[:])

            # ---- acc starts from the center pixel (identity matmul) ----
            acc_ps = [psacc.tile([128, CHUNK], F32, name=f"acc{ch}",
                                 tag=f"acc{ch}")
                      for ch in range(2)]
            for ch in range(2):
                xc = xp_sb[:, 8 * ch : 8 * ch + 8, 3, 0:WO]
                nc.tensor.matmul(
                    acc_ps[ch][:], ident_v, xc,
                    start=True, stop=False, skip_group_check=True,
                )

            # ---- apply: acc += patch_k * (-F_k) for the 9 taps ----
            def acc_mm(prod, last):
                for ch in range(2):
                    nc.tensor.matmul(
                        acc_ps[ch][:],
                        ident_v,
                        prod[:, CHUNK * ch : CHUNK * (ch + 1)],
                        start=False, stop=last, skip_group_check=True,
                    )

            # direct (dx=2) taps first in each group of three: the first
            # DVE multiply can start straight off the first broadcast
            # instead of waiting for an ACT staging copy
            APPLY_ORDER = [0, 1, 2, 3, 4, 5, 6, 7, 8]
            prods = []
            for idx, k in enumerate(APPLY_ORDER):
                dy, dx = k // K, k % K
                ab = 2 * (dy % 2) + (dx % 2)
                ebig = pbig.tile([128, POS], F32, name=f"ebig{k}",
                                 tag="pb")
                for ch in range(2):
                    # row-tiled pair: ch0 on array rows 0..63, ch1 on
                    # 64..127 -> the two matmuls run concurrently
                    nc.tensor.matmul(
                        ebig[:, CHUNK * ch : CHUNK * (ch + 1)],
                        esel_sb[64 * ch : 64 * ch + 64, k, :],
                        f_sb[64 * ch : 64 * ch + 64, :],
                        tile_position=(64 * ch, 0),
                    )
                if dx < 2:
                    # stage Fbig to SBUF bf16 on ScalarE so the DVE multiply
                    # runs in 2x mode (dx=2 taps are misaligned anyway)
                    fb = fb_pool.tile([128, POS], BF16, name=f"fb{k}",
                                      tag="fb")
                    nc.scalar.copy(fb[:], ebig[:])
                    src = fb
                else:
                    src = ebig
                patch = xp_sb[:, dy // 2 : dy // 2 + SUB, ab,
                              dx // 2 : dx // 2 + WO]
                prod = prod_pool.tile([128, POS], BF16, name=f"prod{k}",
                                      tag="prod")
                nc.vector.tensor_mul(
                    prod[:].rearrange("p (r c) -> p r c", r=SUB),
                    patch,
                    src[:].rearrange("p (r c) -> p r c", r=SUB),
                )
                prods.append(prod)
                # lag the accumulation by 2 taps so the PE queue never
                # stalls waiting on the most recent multiply
                if idx >= 2:
                    acc_mm(prods[idx - 2], last=False)
            # finish ch0's accumulator first so its store starts earlier
            for ch in range(2):
                for idx in (K2 - 2, K2 - 1):
                    nc.tensor.matmul(
                        acc_ps[ch][:], ident_v,
                        prods[idx][:, CHUNK * ch : CHUNK * (ch + 1)],
                        start=False, stop=(idx == K2 - 1),
                        skip_group_check=True,
                    )

            # ---- store y = acc (bf16) ----
            y_sb0 = work.tile([128, CHUNK], BF16, tag="ysb0")
            nc.scalar.copy(y_sb0[:], acc_ps[0][:])
            nc.sync.dma_start(y.ap()[:, 0:CHUNK], y_sb0[:])
            y_sb1 = work.tile([128, CHUNK], BF16, tag="ysb1")
            nc.vector.tensor_copy(y_sb1[:], acc_ps[1][:])
            nc.sync.dma_start(y.ap()[:, CHUNK:POS], y_sb1[:])

    nc.compile()
    return nc


def _host_consts(conv_w, gamma, beta, running_mean, running_var):
    scale = gamma / np.sqrt(running_var + EPS)
    shift = beta - running_mean * scale
    w_scaled = conv_w * scale[:, None, None, None]            # [18, 64, 3, 3]

    consts = np.zeros((128, CONST_COLS), np.float32)

    # conv lhsT per tap: block-diag [[W,0],[0,W]], W = w_scaled[o, ci].T
    for dy in range(K):
        for dx in range(K):
            k = 3 * dy + dx
            Wk = w_scaled[:, :, dy, dx]                       # [18, 64]
            blk = np.zeros((128, 64), np.float32)
            blk[0:64, 0:G * K2] = Wk.T
            blk[64:128, 32 : 32 + G * K2] = Wk.T
            consts[:, WTS_C0 + 64 * k : WTS_C0 + 64 * (k + 1)] = blk

    consts[:, IDENT_C0 : IDENT_C0 + 128] = np.eye(128, dtype=np.float32)

    for q in range(4):
        consts[32 * q : 32 * q + G * K2, SEL_C0 + q] = 1.0    # D selector
        consts[q, RSEL_C0 + 32 * q : RSEL_C0 + 32 * (q + 1)] = -1.0

    # selector-generation helpers
    consts[0, BASE_C0 : BASE_C0 + 128] = [
        32 * (j // 64) + K2 * ((j // 32) % 2) for j in range(128)
    ]
    consts[0, ONE_C0 : ONE_C0 + 128] = 1.0

    cb = consts.astype(ml_dtypes.bfloat16)

    # BN shift as raw f32 bytes in two bf16 columns
    bias = np.zeros((128,), np.float32)
    for q in range(4):
        bias[32 * q : 32 * q + G * K2] = shift
    cb[:, BIAS_C0 : BIAS_C0 + 2] = (
        np.frombuffer(bias.astype("<f4").tobytes(), dtype=ml_dtypes.bfloat16)
        .reshape(128, 2)
    )
    pm = (np.arange(128) % 64).astype("<f4")
    cb[:, PM_C0 : PM_C0 + 2] = (
        np.frombuffer(pm.tobytes(), dtype=ml_dtypes.bfloat16).reshape(128, 2)
    )
    return cb


def _host_planes(x):
    """Per-core parity planes [128, 17, 4, 66] bf16 (row-major)."""
    xpad = np.pad(x, ((0, 0), (0, 0), (1, 1), (1, 1)), mode="reflect")
    planes = []
    for core in range(NCORES):
        n, half = core // 2, core % 2
        pl = np.zeros((2, C, PLANE_R, 4, PLANE_W), np.float32)
        for s in range(2):
            r0 = 64 * half + 32 * s
            for a in range(2):
                for b in range(2):
                    sl = xpad[n, :, r0 + a : r0 + 33 : 2, b : 130 : 2]
                    pl[s, :, : sl.shape[1], 2 * a + b, : sl.shape[2]] = sl
        planes.append(
            np.ascontiguousarray(pl.reshape(128, PLANE_R, 4, PLANE_W))
            .astype(ml_dtypes.bfloat16)
        )
    return planes


def _gather_output(results):
    out = np.empty((N, C, HO, WO), np.float32)
    for core, res in enumerate(results):
        n, half = core // 2, core % 2
        yc = np.asarray(res["y"], dtype=np.float32).reshape(2, C, 2, 8, WO)
        # [s, c, ch, r8, w] -> rows 32*half + 16*s + 8*ch + r8
        yc = yc.transpose(1, 0, 2, 3, 4).reshape(C, 32, WO)
        out[n, :, 32 * half : 32 * half + 32, :] = yc
    return out


def _ensure_ntff_hook():
    """Install the axon NTFF profile hook if the image's antenv lacks it."""
    try:
        from antenv import axon_hooks  # noqa: F401
        return
    except ImportError:
        pass
    try:
        import sys
        import types

        import antenv
        from trn_agent_boot.trn_boot import _ntff_profile_via_ctypes

        hook = _ntff_profile_via_ctypes("/opt/axon/libaxon_pjrt.so")
        mod = types.ModuleType("antenv.axon_hooks")
        state = {"hook": hook}
        mod.get_axon_ntff_profile_hook = lambda: state["hook"]
        mod.set_axon_ntff_profile_hook = lambda h: state.update(hook=h)
        sys.modules["antenv.axon_hooks"] = mod
        antenv.axon_hooks = mod
    except Exception:
        pass


def kernel(x, conv_w, gamma, beta, running_mean, running_var):
    global _compiled
    x = np.asarray(x, np.float32)
    conv_w = np.asarray(conv_w, np.float32)
    gamma = np.asarray(gamma, np.float32)
    beta = np.asarray(beta, np.float32)
    running_mean = np.asarray(running_mean, np.float32)
    running_var = np.asarray(running_var, np.float32)

    if _compiled is None:
        _compiled = _build_program()
    nc = _compiled

    cb = _host_consts(conv_w, gamma, beta, running_mean, running_var)
    planes = _host_planes(x)
    in_maps = [{"xplanes": planes[core], "consts": cb}
               for core in range(NCORES)]

    trace = bool(int(os.environ.get("PASA_TRACE", "0")))
    if trace:
        _ensure_ntff_hook()
    res = run_bass_kernel_spmd(
        nc, in_maps, core_ids=list(range(NCORES)), trace=trace
    )
    kernel.last_results = res
    return _gather_output(res.results)


if __name__ == "__main__":
    # quick CoreSim check of core 0 against a numpy re-implementation
    from concourse.bass_interp import CoreSim

    rng = np.random.default_rng(0)
    x = rng.standard_normal((N, C, H, W)).astype(np.float32)
    conv_w = (rng.standard_normal((G * K2, C, K, K))
              * np.sqrt(2.0 / (G * K2 * K * K))).astype(np.float32)
    gamma = rng.uniform(0.5, 1.5, G * K2).astype(np.float32)
    beta = (rng.standard_normal(G * K2) * 0.1).astype(np.float32)
    rmean = (rng.standard_normal(G * K2) * 0.1).astype(np.float32)
    rvar = rng.uniform(0.5, 1.5, G * K2).astype(np.float32)

    nc = _build_program()
    cb = _host_consts(conv_w, gamma, beta, rmean, rvar)
    planes = _host_planes(x)
    sim = CoreSim(nc)
    sim.tensor("xplanes")[:] = planes[0]
    sim.tensor("consts")[:] = cb
    sim.simulate(check_with_hw=False)
    ysim = np.asarray(sim.tensor("y"), dtype=np.float32).reshape(2, C, 2, 8, WO)
    got = ysim.transpose(1, 0, 2, 3, 4).reshape(C, 32, WO)

    # numpy reference for core 0 region (image 0, output rows 0..32)
    scale = gamma / np.sqrt(rvar + EPS)
    shift = beta - rmean * scale
    xpad = np.pad(x[0], ((0, 0), (1, 1), (1, 1)), mode="reflect")
    sig = np.zeros((G * K2, 32, WO), np.float32)
    for o in range(G * K2):
        for dy in range(K):
            for dx in range(K):
                sig[o] += np.einsum(
                    "crw->rw",
                    conv_w[o, :, dy, dx][:, None, None]
                    * xpad[:, dy : dy + 64 : 2, dx : dx + 128 : 2],
                )
    sig = sig * scale[:, None, None] + shift[:, None, None]
    e = np.exp(sig)
    r = 1.0 / e.sum(0)
    accn = np.zeros((C, 32, WO), np.float32)
    for g in range(G):
        for k in range(K2):
            dy, dx = k // K, k % K
            accn[32 * g : 32 * g + 32] += (
                xpad[32 * g : 32 * g + 32, dy : dy + 64 : 2, dx : dx + 128 : 2]
                * e[g * K2 + k][None]
            )
    ref = (xpad[:, 1:65:2, 1:129:2] - accn * r[None]).astype(np.float32)

    err = np.abs(got - ref).max() / np.abs(ref).max()
    print("sim rel err:", err)
